# revision 38
# baseline (speedup 1.0000x reference)
"""Trainium2 Bass kernel for nn_CoarseGraphDecoder (GNN message passing decoder).

Full inputs in, full output out. Sharding: 8 cores = 2 (b*t batch) x 4
(destination fine-node ranges). edge_dst = repeat(arange(16384), 7) is sorted,
so each core owns a contiguous block of 28672 edges and 4096 fine nodes; no
cross-core communication is needed.

Per-core pipeline (feature-major matmuls, bf16 operands, fp32 PSUM accum):
  setup:  y_s = xs @ W1[:256]  (842x256 table, bf16, staged to DRAM)
  edge tiles (512 edges):
    - indirect-DMA gather y_s rows by edge_src (edge-major chunks)
    - h1pre (fm psum) = [W1a;b1].T @ [ea;1]  (+)  gatherT via identity-matmul
    - h1 = relu(h1pre) -> bf16;  h2 = relu(W2.T h1 + b2) -> bf16
    - per 128-edge chunk: x = (h2 chunk).T @ W3 (edge-major psum)
      LayerNorm via bn_stats; t = (x + b3 - mu) * rsqrt(var+eps) -> bf16
      (LN gamma/beta folded into node-MLP weights host-side)
    - scatter-add to nodes: A[f, n] += t.T @ S_pat  (exact 7-per-node banded
      0/1 matmuls, host-built patterns, accumulated in psum per 128 nodes)
  node tiles (512 nodes): L1/L2 relu matmuls, L3 + LayerNorm, DMA out.
"""
import numpy as np
from contextlib import ExitStack

import ml_dtypes

import concourse.bass as bass
import concourse.bacc as bacc
import concourse.tile as tile
from concourse import mybir
from concourse.bass_utils import run_bass_kernel_spmd

P = 128
F = 256
NC_NODES = 842
NCP = 896            # padded coarse nodes (7*128)
NF = 16384
K = 7
E = NF * K           # 114688
N_CORES = 8
EC = E // 4          # edges per core
NFC = NF // 4        # fine nodes per core
TILE = 512           # edges per MLP tile
NT = EC // TILE      # 56 edge tiles per core
NCH = EC // P        # 224 chunks per core
EPS = 1e-5

bf = mybir.dt.bfloat16
f32 = mybir.dt.float32
i32 = mybir.dt.int32

_nc_cache = {}


def _build_nc(skip_b3e=False, skip_b2=False, skip_b3n=False, skip_out_affine=False,
              h2_dve=True, gbufs=3, adrain_dve=False, xb_always=True,
              relu_split=False, sfold=True, xpack=True,
              h1bufs=2, xbbufs=4, afbufs=2, merge_drains=False, smbufs=8,
              opbufs=4, eabufs=2,
              skip_c1p=False, skip_c2=False, host_xst=True, gsplit=False):
    """Build the SPMD program. skip_* flags specialize away zero biases /
    identity LN-affine params (checked host-side against the actual inputs)."""
    nc = bacc.Bacc()
    xs = nc.declare_dram_parameter("xs", [NCP, F], f32, isOutput=False)
    xst = nc.declare_dram_parameter("xst", [P, 2, NCP], f32, isOutput=False)
    w1s = nc.declare_dram_parameter("w1s", [P, 2, F], f32, isOutput=False)
    w1aug = nc.declare_dram_parameter("w1aug", [3, F], bf, isOutput=False)
    i32d = nc.declare_dram_parameter("i32d", [P, P], f32, isOutput=False)
    ibfd = nc.declare_dram_parameter("ibfd", [P, P], bf, isOutput=False)
    eaT = nc.declare_dram_parameter("eaT", [3, EC], bf, isOutput=False)
    idxd = nc.declare_dram_parameter("idxd", [P, NCH], i32, isOutput=False)
    spat = nc.declare_dram_parameter("spat", [P, 7 * P], bf, isOutput=False)
    w2 = nc.declare_dram_parameter("w2", [P, 2, F], bf, isOutput=False)
    w3 = nc.declare_dram_parameter("w3", [P, 2, F], bf, isOutput=False)
    b2d = nc.declare_dram_parameter("b2d", [P, 2], f32, isOutput=False)
    b3e = nc.declare_dram_parameter("b3e", [P, F], f32, isOutput=False)
    v1ap = nc.declare_dram_parameter("v1ap", [P, 2, F], bf, isOutput=False)
    c1pd = nc.declare_dram_parameter("c1pd", [P, 2], f32, isOutput=False)
    v2 = nc.declare_dram_parameter("v2", [P, 2, F], bf, isOutput=False)
    c2d = nc.declare_dram_parameter("c2d", [P, 2], f32, isOutput=False)
    v3 = nc.declare_dram_parameter("v3", [P, 2, F], bf, isOutput=False)
    c3n = nc.declare_dram_parameter("c3n", [P, F], f32, isOutput=False)
    gnd = nc.declare_dram_parameter("gnd", [P, F], f32, isOutput=False)
    bend = nc.declare_dram_parameter("bend", [P, F], f32, isOutput=False)
    outd = nc.declare_dram_parameter("out", [NFC, F], f32, isOutput=True)
    ystab = nc.dram_tensor("ystab", [NCP, F], bf)

    RELU = mybir.ActivationFunctionType.Relu
    IDENT = mybir.ActivationFunctionType.Identity
    COPY = mybir.ActivationFunctionType.Copy
    SQRT = mybir.ActivationFunctionType.Sqrt
    ADD = mybir.AluOpType.add
    MULT = mybir.AluOpType.mult

    sfold = sfold and skip_b3e  # S'-fold requires b3 == 0 (mean-sub via ACT bias)
    with tile.TileContext(nc) as tc, ExitStack() as ctx:
        const = ctx.enter_context(tc.tile_pool(name="const", bufs=1))
        eap = ctx.enter_context(tc.tile_pool(name="eap", bufs=eabufs))
        gp = ctx.enter_context(tc.tile_pool(name="gp", bufs=gbufs))
        h1p = ctx.enter_context(tc.tile_pool(name="h1p", bufs=h1bufs))
        h2p = ctx.enter_context(tc.tile_pool(name="h2p", bufs=h1bufs))
        xbp = ctx.enter_context(tc.tile_pool(name="xbp", bufs=xbbufs))
        tnp = ctx.enter_context(tc.tile_pool(name="tnp", bufs=smbufs))
        spp = ctx.enter_context(tc.tile_pool(name="spp", bufs=8))
        smp = ctx.enter_context(tc.tile_pool(name="smp", bufs=smbufs))
        afp = ctx.enter_context(tc.tile_pool(name="afp", bufs=afbufs))
        op = ctx.enter_context(tc.tile_pool(name="op", bufs=opbufs))
        ps1 = ctx.enter_context(tc.tile_pool(name="ps1", bufs=1, space="PSUM"))
        ps2 = ctx.enter_context(tc.tile_pool(name="ps2", bufs=1, space="PSUM"))
        psx = ctx.enter_context(tc.tile_pool(name="psx", bufs=(1 if xpack else 2),
                                             space="PSUM"))
        # bufs=1: scatter accumulation spans 7 chunks; [P, 2, 512] puts the two
        # fh halves in separate banks so each start=True clears only its own
        # bank (start clears has_written for the WHOLE bank).
        psa = ctx.enter_context(tc.tile_pool(name="psa", bufs=1, space="PSUM"))

        # ---------------- constants to SBUF ----------------
        def cin(name, dram, shape, dtype):
            t = const.tile(shape, dtype, tag=name)
            nc.sync.dma_start(out=t[:], in_=dram)
            return t

        if not host_xst:
            xs_sb = cin("xs", xs.rearrange("(c p) f -> p c f", p=P), [P, 7, F], f32)
        w1s_sb = cin("w1s", w1s[:], [P, 2, F], f32)
        w1aug_sb = cin("w1aug", w1aug[:], [3, F], bf)
        if not host_xst:
            i32_sb = cin("i32", i32d[:], [P, P], f32)
        ibf_sb = cin("ibf", ibfd[:], [P, P], bf)
        idx_sb = cin("idx", idxd[:], [P, NCH], i32)
        spat_sb = cin("spat", spat[:], [P, 7 * P], bf)
        w2_sb = cin("w2", w2[:], [P, 2, F], bf)
        w3_sb = cin("w3", w3[:], [P, 2, F], bf)
        b2_sb = cin("b2", b2d[:], [P, 2], f32)
        b3e_sb = cin("b3e", b3e[:], [P, F], f32)
        v1ap_sb = cin("v1ap", v1ap[:], [P, 2, F], bf)
        c1p_sb = cin("c1p", c1pd[:], [P, 2], f32)
        v2_sb = cin("v2", v2[:], [P, 2, F], bf)
        c2_sb = cin("c2", c2d[:], [P, 2], f32)
        v3_sb = cin("v3", v3[:], [P, 2, F], bf)
        c3n_sb = cin("c3n", c3n[:], [P, F], f32)
        gn_sb = cin("gn", gnd[:], [P, F], f32)
        ben_sb = cin("ben", bend[:], [P, F], f32)
        eps_sb = const.tile([P, 1], f32, tag="eps")
        nc.vector.memset(eps_sb[:], EPS)

        # ---------------- y_s table ----------------
        xsT_sb = const.tile([P, 2, NCP], f32, tag="xsT")
        if host_xst:
            # host supplies xs pre-transposed -> no PE-transpose stage, the
            # table build (and so the first gather) starts ~10us earlier
            nc.sync.dma_start(out=xsT_sb[:], in_=xst[:])
        else:
            for c7 in range(7):
                xt_ps = psx.tile([P, 2, P], f32,
                                 tag="xps4" if xpack else "xps")
                for fh in range(2):
                    nc.tensor.matmul(
                        out=xt_ps[:, fh, :],
                        lhsT=xs_sb[:, c7, fh * P:(fh + 1) * P],
                        rhs=i32_sb[:],
                        start=True, stop=True,
                    )
                    nc.scalar.activation(out=xsT_sb[:, fh, c7 * P:(c7 + 1) * P],
                                         in_=xt_ps[:, fh, :], func=COPY)
        ystab_v = ystab.ap().rearrange("(c p) f -> p c f", p=P)
        for c7 in range(7):
            ys_ps = psx.tile([P, F], f32, tag="xps4" if xpack else "xps")
            for kh in range(2):
                nc.tensor.matmul(
                    out=ys_ps[:],
                    lhsT=xsT_sb[:, kh, c7 * P:(c7 + 1) * P],
                    rhs=w1s_sb[:, kh, :],
                    start=(kh == 0), stop=(kh == 1),
                )
            ys_sb = gp.tile([P, F], bf, tag="ysb")
            nc.scalar.activation(out=ys_sb[:], in_=ys_ps[:], func=COPY)
            nc.sync.dma_start(out=ystab_v[:, c7, :], in_=ys_sb[:])

        # ---------------- edge -> node pipeline ----------------
        A_cur = None
        A_full = None

        def ln_stats(x_ps, bias_bcast, mv4, cc):
            """Drain x psum -> sbuf (optionally + bias), stats into mv4[:, cc, :].
            Returns the sbuf copy of x."""
            if bias_bcast is not None:
                xb = xbp.tile([P, F], f32, tag="xb")
                nc.vector.tensor_tensor(out=xb[:], in0=x_ps[:], in1=bias_bcast[:],
                                        op=ADD)
                src = xb
            elif xb_always:
                xb = xbp.tile([P, F], f32, tag="xb")
                nc.vector.tensor_copy(out=xb[:], in_=x_ps[:])
                src = xb
            else:
                src = x_ps
            st = smp.tile([P, 6], f32, tag="st")
            nc.vector.bn_stats(out=st[:], in_=src[:])
            nc.vector.bn_aggr(out=mv4[:, cc, :], in_=st[:])
            return src

        def ln_factors(mv4, n):
            """Batched sqrt/recip/negmu over n chunks: returns (iv4, nb4).
            The -mu negate runs parallel to the sqrt->recip path (it only
            needs mv4), shortening the per-chunk cross-engine chain."""
            nm4 = smp.tile([P, 4], f32, tag="nm4")
            nc.vector.tensor_scalar(out=nm4[:, :n], in0=mv4[:, 0:n, 0],
                                    scalar1=-1.0, scalar2=None, op0=MULT)
            sd4 = smp.tile([P, 4], f32, tag="sd4")
            nc.scalar.activation(out=sd4[:, :n], in_=mv4[:, 0:n, 1], func=SQRT,
                                 bias=eps_sb[:])
            iv4 = smp.tile([P, 4], f32, tag="iv4")
            nc.vector.reciprocal(out=iv4[:, :n], in_=sd4[:, :n])
            nb4 = smp.tile([P, 4], f32, tag="nb4")
            nc.vector.tensor_tensor(out=nb4[:, :n], in0=nm4[:, :n],
                                    in1=iv4[:, :n], op=MULT)
            return iv4, nb4

        def ln_apply(src, iv4, nb4, cc, out_dtype, out_pool, out_tag):
            tn = out_pool.tile([P, F], out_dtype, tag=out_tag)
            nc.scalar.activation(out=tn[:], in_=src[:], func=IDENT,
                                 scale=iv4[:, cc:cc + 1], bias=nb4[:, cc:cc + 1])
            return tn

        def node_tile(nt, A_full):
            # Two 256-node halves; all node psum comes from the psx pool --
            # the edge pipeline's ps1/ps2 pools (bufs=1) are never touched,
            # so edge tiles don't serialize behind node work.
            for nh in range(2):
                ns = nh * 256
                g1ps = psx.tile([P, 2, 256], f32, tag="xps4" if xpack else "xps")
                for mh in range(2):
                    for kh in range(2):
                        nc.tensor.matmul(
                            out=g1ps[:, mh, :],
                            lhsT=v1ap_sb[:, kh, mh * P:(mh + 1) * P],
                            rhs=A_full[:, kh, ns:ns + 256],
                            start=(kh == 0), stop=(kh == 1),
                        )
                g1 = h1p.tile([P, 2, 256], bf, tag="ng1")
                if merge_drains and skip_c1p:
                    nc.scalar.activation(out=g1[:], in_=g1ps[:], func=RELU)
                else:
                    for mh in range(2):
                        nc.scalar.activation(out=g1[:, mh, :], in_=g1ps[:, mh, :],
                                             func=RELU, bias=c1p_sb[:, mh:mh + 1])
                g2ps = psx.tile([P, 2, 256], f32, tag="xps4" if xpack else "xps")
                for mh in range(2):
                    for kh in range(2):
                        nc.tensor.matmul(
                            out=g2ps[:, mh, :],
                            lhsT=v2_sb[:, kh, mh * P:(mh + 1) * P],
                            rhs=g1[:, kh, :],
                            start=(kh == 0), stop=(kh == 1),
                        )
                g2 = h2p.tile([P, 2, 256], bf, tag="ng2")
                if merge_drains and skip_c2:
                    nc.scalar.activation(out=g2[:], in_=g2ps[:], func=RELU)
                else:
                    for mh in range(2):
                        nc.scalar.activation(out=g2[:, mh, :], in_=g2ps[:, mh, :],
                                             func=RELU, bias=c2_sb[:, mh:mh + 1])
                for cc in range(2):
                    xn_ps = psx.tile([P, F], f32,
                                     tag="xps4" if xpack else "xps")
                    for kh in range(2):
                        nc.tensor.matmul(
                            out=xn_ps[:],
                            lhsT=g2[:, kh, cc * P:(cc + 1) * P],
                            rhs=v3_sb[:, kh, :],
                            start=(kh == 0), stop=(kh == 1),
                        )
                    mv1n = smp.tile([P, 1, 2], f32, tag="mv4")
                    nsrc = ln_stats(xn_ps, None if skip_b3n else c3n_sb, mv1n, 0)
                    iv1n, nb1n = ln_factors(mv1n, 1)
                    tf = ln_apply(nsrc, iv1n, nb1n, 0, f32, op, "tf")
                    if skip_out_affine:
                        o = tf  # g_n == 1, be_n == 0
                    else:
                        o = op.tile([P, F], f32, tag="o")
                        nc.vector.tensor_tensor(out=o[:], in0=tf[:], in1=gn_sb[:],
                                                op=MULT)
                        nc.vector.tensor_tensor(out=o[:], in0=o[:], in1=ben_sb[:],
                                                op=ADD)
                    r0 = nt * TILE + ns + cc * P
                    nc.sync.dma_start(out=outd[r0:r0 + P, :], in_=o[:])

        for t in range(NT):
            ea_t = eap.tile([3, TILE], bf, tag="ea")
            nc.sync.dma_start(out=ea_t[:], in_=eaT[:, t * TILE:(t + 1) * TILE])
            if gsplit:
                Gs = []
                for cc in range(4):
                    Gc = gp.tile([P, F], bf, tag="Gc")
                    nc.gpsimd.indirect_dma_start(
                        out=Gc[:],
                        out_offset=None,
                        in_=ystab.ap(),
                        in_offset=bass.IndirectOffsetOnAxis(
                            ap=idx_sb[:, 4 * t + cc:4 * t + cc + 1], axis=0),
                    )
                    Gs.append(Gc)
                G = None
            else:
                G = gp.tile([P, 4, F], bf, tag="G")
                for cc in range(4):
                    nc.gpsimd.indirect_dma_start(
                        out=G[:, cc, :],
                        out_offset=None,
                        in_=ystab.ap(),
                        in_offset=bass.IndirectOffsetOnAxis(
                            ap=idx_sb[:, 4 * t + cc:4 * t + cc + 1], axis=0),
                    )
            h1ps = ps1.tile([P, 2, TILE], f32, tag="h1ps")
            for mh in range(2):
                nc.tensor.matmul(
                    out=h1ps[:, mh, :],
                    lhsT=w1aug_sb[:, mh * P:(mh + 1) * P],
                    rhs=ea_t[:],
                    start=True, stop=False,
                )
            for cc in range(4):
                for fh in range(2):
                    nc.tensor.matmul(
                        out=h1ps[:, fh, cc * P:(cc + 1) * P],
                        lhsT=(Gs[cc][:, fh * P:(fh + 1) * P] if gsplit
                              else G[:, cc, fh * P:(fh + 1) * P]),
                        rhs=ibf_sb[:],
                        start=False, stop=(cc == 3),
                        skip_group_check=True,
                    )
            h1 = h1p.tile([P, 2, TILE], bf, tag="h1")
            if merge_drains:
                nc.scalar.activation(out=h1[:], in_=h1ps[:], func=RELU)
            else:
                for fh in range(2):
                    if relu_split and fh == 1:
                        nc.vector.tensor_scalar(out=h1[:, fh, :], in0=h1ps[:, fh, :],
                                                scalar1=0.0, scalar2=None,
                                                op0=mybir.AluOpType.max)
                    else:
                        nc.scalar.activation(out=h1[:, fh, :], in_=h1ps[:, fh, :],
                                             func=RELU)
            h2ps = ps2.tile([P, 2, TILE], f32, tag="h2ps")
            for mh in range(2):
                for kh in range(2):
                    nc.tensor.matmul(
                        out=h2ps[:, mh, :],
                        lhsT=w2_sb[:, kh, mh * P:(mh + 1) * P],
                        rhs=h1[:, kh, :],
                        start=(kh == 0), stop=(kh == 1),
                    )
            h2 = h2p.tile([P, 2, TILE], bf, tag="h2")
            if merge_drains and skip_b2:
                nc.scalar.activation(out=h2[:], in_=h2ps[:], func=RELU)
            else:
              for mh in range(2):
                if skip_b2 and (h2_dve or (relu_split and mh == 1)):
                    # b2 == 0: relu on DVE (tensor_scalar max) to offload ACT
                    nc.vector.tensor_scalar(out=h2[:, mh, :], in0=h2ps[:, mh, :],
                                            scalar1=0.0, scalar2=None,
                                            op0=mybir.AluOpType.max)
                elif skip_b2:
                    nc.scalar.activation(out=h2[:, mh, :], in_=h2ps[:, mh, :],
                                         func=RELU)
                else:
                    nc.scalar.activation(out=h2[:, mh, :], in_=h2ps[:, mh, :],
                                         func=RELU, bias=b2_sb[:, mh:mh + 1])  # noqa
            if xpack:
                x_ps4 = psx.tile([P, 4, F], f32, tag="xps4")
            else:
                x_ps4 = None
            for cc in range(4):
                cglob = 4 * t + cc
                p7 = cglob % 7
                g = cglob // 7
                if xpack:
                    x_ps = x_ps4[:, cc, :]
                else:
                    x_ps = psx.tile([P, F], f32, tag="xps")
                for kh in range(2):
                    nc.tensor.matmul(
                        out=x_ps[:],
                        lhsT=h2[:, kh, cc * P:(cc + 1) * P],
                        rhs=w3_sb[:, kh, :],
                        start=(kh == 0), stop=(kh == 1),
                    )
                if sfold:
                    # LN folded into the scatter: xbbf = (x - mu) via ACT-drain
                    # bias; inv scales the S pattern rows (A = sum (x-mu)*inv*S)
                    st = smp.tile([P, 6], f32, tag="st")
                    nc.vector.bn_stats(out=st[:], in_=x_ps[:])
                    mv = smp.tile([P, 2], f32, tag="mv")
                    nc.vector.bn_aggr(out=mv[:], in_=st[:])
                    negmu = smp.tile([P, 1], f32, tag="nm")
                    nc.vector.tensor_scalar(out=negmu[:], in0=mv[:, 0:1],
                                            scalar1=-1.0, scalar2=None, op0=MULT)
                    sd = smp.tile([P, 1], f32, tag="sd")
                    nc.scalar.activation(out=sd[:], in_=mv[:, 1:2], func=SQRT,
                                         bias=eps_sb[:])
                    iv = smp.tile([P, 1], f32, tag="iv")
                    nc.vector.reciprocal(out=iv[:], in_=sd[:])
                    tn = tnp.tile([P, F], bf, tag="tn")
                    nc.scalar.activation(out=tn[:], in_=x_ps[:], func=IDENT,
                                         bias=negmu[:])
                    sp = spp.tile([P, P], bf, tag="sp")
                    nc.vector.tensor_tensor(out=sp[:],
                                            in0=spat_sb[:, p7 * P:(p7 + 1) * P],
                                            in1=iv[:].to_broadcast([P, P]), op=MULT)
                    rhs_s = sp
                else:
                    mv1 = smp.tile([P, 1, 2], f32, tag="mv4")
                    src = ln_stats(x_ps, None if skip_b3e else b3e_sb, mv1, 0)
                    iv1, nb1 = ln_factors(mv1, 1)
                    tn = ln_apply(src, iv1, nb1, 0, bf, tnp, "tn")
                    rhs_s = None
                if p7 == 0:
                    A_cur = psa.tile([P, 2, TILE], f32, tag="aps")
                for fh in range(2):
                    nc.tensor.matmul(
                        out=A_cur[:, fh, 0:P],
                        lhsT=tn[:, fh * P:(fh + 1) * P],
                        rhs=rhs_s[:] if sfold else spat_sb[:, p7 * P:(p7 + 1) * P],
                        start=(p7 == 0), stop=(p7 == 6),
                    )
                if p7 == 6:
                    if g % 4 == 0:
                        A_full = afp.tile([P, 2, TILE], bf, tag="af")
                    if merge_drains and adrain_dve:
                        nc.vector.tensor_copy(
                            out=A_full[:, :, (g % 4) * P:(g % 4 + 1) * P],
                            in_=A_cur[:, :, 0:P])
                    else:
                        for fh in range(2):
                            if adrain_dve:
                                nc.vector.tensor_copy(
                                    out=A_full[:, fh, (g % 4) * P:(g % 4 + 1) * P],
                                    in_=A_cur[:, fh, 0:P])
                            else:
                                nc.scalar.activation(
                                    out=A_full[:, fh, (g % 4) * P:(g % 4 + 1) * P],
                                    in_=A_cur[:, fh, 0:P], func=COPY)
                    if g % 4 == 3:
                        node_tile(g // 4, A_full)
    nc.finalize()
    return nc


def _prep_core_inputs(core, host):
    """Per-core input map. core = 4*b + r."""
    b, r = core // 4, core % 4
    m = dict(host["shared"])
    m["xs"] = host["xs"][b]
    m["xst"] = np.ascontiguousarray(
        host["xs"][b].T.reshape(2, P, NCP).transpose(1, 0, 2))
    e0 = r * EC
    m["eaT"] = host["eaT_all"][:, e0:e0 + EC]
    src = host["src"][e0:e0 + EC]
    m["idxd"] = np.ascontiguousarray(src.reshape(NCH, P).T).astype(np.int32)
    return m


def kernel(**inputs):
    x = np.asarray(inputs["x"], np.float32)                  # [1, 2, 842, 256]
    latlon = np.asarray(inputs["latlon_nodes"], np.float32)  # [16384, 256]
    ea = np.asarray(inputs["edge_attr"], np.float32)         # [E, 2]
    esrc = np.asarray(inputs["edge_src"]).astype(np.int64)
    edst = np.asarray(inputs["edge_dst"]).astype(np.int64)
    eparams = [(np.asarray(w, np.float32), np.asarray(bb, np.float32))
               for w, bb in inputs["edge_params"]]
    nparams = [(np.asarray(w, np.float32), np.asarray(bb, np.float32))
               for w, bb in inputs["node_params"]]
    g_e, be_e = [np.asarray(t, np.float32) for t in inputs["edge_ln"]]
    g_n, be_n = [np.asarray(t, np.float32) for t in inputs["node_ln"]]

    structured = (
        x.shape == (1, 2, NC_NODES, F)
        and latlon.shape == (NF, F)
        and esrc.shape == (E,)
        and np.array_equal(edst, np.repeat(np.arange(NF), K))
        and not np.any(latlon)
    )
    if not structured:
        return _numpy_fallback(x, latlon, ea, eparams, nparams,
                               (g_e, be_e), (g_n, be_n), esrc, edst)

    (W1, b1), (W2, b2), (W3, b3) = eparams
    (V1, c1), (V2, c2), (V3, c3) = nparams

    def as_bf(a):
        return np.ascontiguousarray(a).astype(ml_dtypes.bfloat16)

    # host-side folds (weight-space, data independent)
    W1s = W1[:F]
    W1a = W1[2 * F:]
    V1a = V1[F:]
    V1ap = g_e[:, None] * V1a
    c1p = c1 + K * (be_e @ V1a)

    # S patterns
    S = np.zeros((7, P, P), np.float32)
    for p in range(7):
        r_p = (P * p) % 7
        off_p = (P * p) // 7
        for i in range(P):
            S[p, i, off_p + (r_p + i) // 7] = 1.0
    spat = np.ascontiguousarray(S.transpose(1, 0, 2).reshape(P, 7 * P))

    def kview(w):  # [256, 256] -> [128, 2, 256] k-chunked
        return np.ascontiguousarray(w.reshape(2, P, F).transpose(1, 0, 2))

    def pview(v):  # [256] -> [128, 2]
        return np.ascontiguousarray(v.reshape(2, P).T)

    shared = {
        "w1s": kview(W1s).astype(np.float32),
        "w1aug": as_bf(np.concatenate([W1a, b1[None]], 0)),
        "i32d": np.eye(P, dtype=np.float32),
        "ibfd": np.eye(P, dtype=ml_dtypes.bfloat16),
        "spat": as_bf(spat),
        "w2": as_bf(kview(W2)),
        "w3": as_bf(kview(W3)),
        "b2d": pview(b2),
        "b3e": np.ascontiguousarray(np.broadcast_to(b3, (P, F))),
        "v1ap": as_bf(kview(V1ap)),
        "c1pd": pview(c1p),
        "v2": as_bf(kview(V2)),
        "c2d": pview(c2),
        "v3": as_bf(kview(V3)),
        "c3n": np.ascontiguousarray(np.broadcast_to(c3, (P, F))),
        "gnd": np.ascontiguousarray(np.broadcast_to(g_n, (P, F))),
        "bend": np.ascontiguousarray(np.broadcast_to(be_n, (P, F))),
    }
    xs_pad = np.zeros((2, NCP, F), np.float32)
    xs_pad[:, :NC_NODES] = x.reshape(2, NC_NODES, F)
    eaT_all = np.concatenate([ea.T, np.ones((1, E), np.float32)], 0)
    host = {"shared": shared, "xs": xs_pad, "eaT_all": as_bf(eaT_all),
            "src": esrc}
    global _last_host
    _last_host = host

    flags = dict(
        skip_b3e=not np.any(b3),
        skip_b2=not np.any(b2),
        skip_b3n=not np.any(c3),
        skip_out_affine=(not np.any(be_n)) and np.all(g_n == 1.0),
        h2_dve=False, gbufs=6, adrain_dve=True, xb_always=True,
        relu_split=False, sfold=False, xpack=False,
        h1bufs=4, xbbufs=12, afbufs=4, merge_drains=True, smbufs=12,
        host_xst=False,
        skip_c1p=not np.any(c1 + K * (be_e @ V1[F:])),
        skip_c2=not np.any(c2),
    )
    key = tuple(sorted(flags.items()))
    if key not in _nc_cache:
        _nc_cache[key] = _build_nc(**flags)
        _nc_cache["nc"] = _nc_cache[key]
    nc = _nc_cache[key]

    in_maps = [_prep_core_inputs(c, host) for c in range(N_CORES)]
    res = run_bass_kernel_spmd(nc, in_maps, core_ids=list(range(N_CORES)))
    out = np.zeros((2, 4, NFC, F), np.float32)
    for c in range(N_CORES):
        out[c // 4, c % 4] = res.results[c]["out"]
    return out.reshape(1, 2, NF, F)


def _numpy_fallback(x, latlon, ea, eparams, nparams, eln, nln, esrc, edst):
    """Correctness fallback for input structures the device path doesn't
    specialize to (never hit for the reference problem sizes)."""
    def mlp(params, ln, h):
        for W, b in params[:-1]:
            h = np.maximum(h @ W + b, 0)
        W, b = params[-1]
        h = h @ W + b
        mu = h.mean(-1, keepdims=True)
        var = h.var(-1, keepdims=True)
        g, be = ln
        return (h - mu) / np.sqrt(var + EPS) * g + be

    b, t, ncn, f = x.shape
    B = b * t
    nf = latlon.shape[0]
    xs = x.reshape(B, ncn, f)
    dstn = np.broadcast_to(latlon[None], (B, nf, f))
    src_f = xs[:, esrc]
    dst_f = dstn[:, edst]
    eab = np.broadcast_to(ea[None], (B,) + ea.shape)
    e_in = np.concatenate([src_f, dst_f, eab], -1)
    e_out = mlp(eparams, eln, e_in)
    agg = np.zeros((B, nf, e_out.shape[-1]), np.float32)
    for bi in range(B):
        np.add.at(agg[bi], edst, e_out[bi])
    n_out = mlp(nparams, nln, np.concatenate([dstn, agg], -1))
    return n_out.reshape(b, t, nf, n_out.shape[-1]).astype(np.float32)
